# revision 4
# baseline (speedup 1.0000x reference)
"""BiLSTM (B=16, T=2048, D=U=256) on 8 TRN2 NeuronCores.

Sharding: 8 cores = 2 directions x 4 batch-shards (B_local=4 per core).
Backward cores receive x time-reversed on the host; all cores run the same
SPMD program (a forward scan), so no collectives are needed.  Keras-style
go_backwards semantics mean the backward half is emitted in iteration
order, which is exactly the scan order on the backward cores.

Per-core kernel: precompute xw[t] = x_t @ W on the TensorEngine (bf16,
gate order [cand i f o], candidate columns pre-doubled on the host so
tanh(x) = 2*sigmoid(2x)-1 needs only the sigmoid table), then run the
2048-step recurrence.  Per step the PE accumulates 16 R-tile matmuls
(R and h in float8e4 — halves the per-step LDWEIGHTS stream; validated
rel err ~5e-3) on top of identity-matmuls that inject xw_t into PSUM,
with the [cand,i,f] and [o] gates in separate PSUM banks so ScalarE's
batched sigmoid over [cand|i|f] can start while the o-chunk matmuls
finish.  VectorE does the cell update (cand affine, one fused
[i|f]*[cand|c] multiply, pair add); tanh(sigmoid(s)) is approximated as
K*sigmoid(AL*s + BE) (max err 8.6e-4) so the output nonlinearity is a
single ScalarE op with K folded into R and the output copy, and
c' = sigmoid(s) runs off the critical path.  h is written twice: fp8 for
the recurrence, bf16 for the staged f32 output.
"""

import numpy as np

F32 = None  # set on first build

_CACHE = {}

T = 2048
B = 16
D = 256
U = 256
G = 4 * U
BL = 4  # batch per core

K_PHI = 0.7589144336406901
AL_PHI = 1.0834263081088795
BE_PHI = 0.44379053813456204


def _patch_tile_drain():
    """This container's walrus accepts only one sem-wait/update per
    instruction; spread Tile's final-drain waits across NOPs."""
    import concourse.tile as tile
    import concourse.mybir as mybir
    from concourse.vector_clock import ScopedClock

    if getattr(tile.TileContext, "_lstm_patched", False):
        return

    def _drain_and_barrier(self, tick_clock, wait_clock):
        carrier = self.nc.sync.nop(nofuse=True, hint="final_wait_carrier")
        wait_clock.add_sem_waits(
            carrier.ins, ScopedClock({None: tick_clock.global_clock})
        )
        si = carrier.ins.sync_info
        waits = list(si.on_wait or []) if si is not None else []
        if len(waits) > 1:
            si.on_wait = waits[:1]
            for wx in waits[1:]:
                n = self.nc.sync.nop(nofuse=True, hint="final_wait_extra")
                if n.ins.sync_info is None:
                    n.ins.sync_info = mybir.SyncInfo(on_wait=[wx], on_update=[])
                else:
                    n.ins.sync_info.on_wait = [wx]
        self.nc.sync.drain()
        self.nc.all_engine_barrier()
        assert self.sems is not None
        popped = self.nc._tile_sem_poison_stack.pop()
        assert popped is self._sem_poison
        self.nc.clear_and_free_semaphores(list(self.sems.allocated().values()))
        self.nc.all_engine_barrier()

    tile.TileContext._drain_and_barrier = _drain_and_barrier
    tile.TileContext._lstm_patched = True


def _split_syncs(nc, max_waits=1, max_updates=1):
    import concourse.mybir as mybir

    ctr = [0]

    def mknop(engine, waits, updates):
        ctr[0] += 1
        return mybir.InstNoOp(
            name=f"syncfix-{ctr[0]}",
            engine=engine,
            sync_info=mybir.SyncInfo(on_wait=list(waits), on_update=list(updates)),
        )

    for f in nc.m.functions:
        for bb in f.blocks:
            changed = False
            out = []
            for inst in bb.instructions:
                si = inst.sync_info
                if si is None or inst.engine == mybir.EngineType.Unassigned:
                    out.append(inst)
                    continue
                waits = list(si.on_wait or [])
                updates = list(si.on_update or [])
                if len(waits) <= max_waits and len(updates) <= max_updates:
                    out.append(inst)
                    continue
                changed = True
                for wx in waits[:-max_waits] if max_waits else waits:
                    out.append(mknop(inst.engine, [wx], []))
                si.on_wait = waits[-max_waits:] if max_waits else []
                extra_u = updates[max_updates:] if max_updates else updates
                si.on_update = updates[:max_updates] if max_updates else []
                out.append(inst)
                for ux in extra_u:
                    out.append(mknop(inst.engine, [], [ux]))
            if changed:
                bb.instructions = out
    return nc


def _build_v3(seg=128, proj_tb=128, split_sig=True, fp8=True, use_tanh=False,
              hfull=True, B=BL):
    import concourse.bass as bass
    import concourse.mybir as mybir
    import concourse.tile as tile
    from contextlib import ExitStack

    _patch_tile_drain()
    F32 = mybir.dt.float32
    BF16 = mybir.dt.bfloat16
    FP8 = mybir.dt.float8e4
    SIG = mybir.ActivationFunctionType.Sigmoid
    COPY = mybir.ActivationFunctionType.Copy
    nc = bass.Bass()
    xt = nc.dram_tensor("xt", [2, 128, T * B], F32, kind="ExternalInput")
    w = nc.dram_tensor("w", [D, G], F32, kind="ExternalInput")
    r = nc.dram_tensor("r", [U, G], F32, kind="ExternalInput")
    bcg = nc.dram_tensor("bcg", [128, 2], F32, kind="ExternalInput")
    out = nc.dram_tensor("out", [2, 128, T * B], F32, kind="ExternalOutput")

    RDT = FP8 if fp8 else BF16
    NB = B
    HW = 2 * NB
    W8 = 8 * NB

    with ExitStack() as ctx:
        tc = ctx.enter_context(tile.TileContext(nc))
        const = ctx.enter_context(tc.tile_pool(name="const", bufs=1))
        big = ctx.enter_context(tc.tile_pool(name="big", bufs=1))
        wstage = ctx.enter_context(tc.tile_pool(name="wstage", bufs=2))
        xload = ctx.enter_context(tc.tile_pool(name="xload", bufs=2))
        xcast = ctx.enter_context(tc.tile_pool(name="xcast", bufs=2))
        ppsum = ctx.enter_context(tc.tile_pool(name="ppsum", bufs=2, space="PSUM"))
        gpsum = ctx.enter_context(tc.tile_pool(name="gpsum", bufs=2, space="PSUM"))
        work = ctx.enter_context(tc.tile_pool(name="work", bufs=3))
        hsegp = ctx.enter_context(tc.tile_pool(name="hsegp", bufs=2))
        ostage = ctx.enter_context(tc.tile_pool(name="ostage", bufs=2))

        wb = const.tile([128, 2, G], BF16)
        rb = const.tile([128, 2, G], RDT)
        bct = const.tile([128, 2], F32)
        ident = const.tile([128, 128], BF16)
        hzero = const.tile([128, HW], RDT)
        bphi = const.tile([128, 1], F32)
        nc.vector.memset(bphi[:, :], BE_PHI)

        for src, dst in ((w, wb), (r, rb)):
            for k in range(2):
                st = wstage.tile([128, G], F32, tag="wst")
                nc.sync.dma_start(out=st[:, :], in_=src[k * 128:(k + 1) * 128, :])
                nc.scalar.copy(dst[:, k, :], st[:, :])
        nc.sync.dma_start(out=bct[:, :], in_=bcg[:, :])
        from concourse.masks import make_identity
        make_identity(nc, ident[:, :])
        nc.vector.memset(hzero[:, :], 0.0)

        xw = big.tile([128, T, W8], BF16)
        if hfull:
            h2 = big.tile([128, T, HW], RDT)
        else:
            h2 = big.tile([128, 2, HW], RDT)  # fp8 h/K ping-pong

        ntb = T // proj_tb
        ntok = proj_tb * B
        for tb in range(ntb):
            t0 = tb * proj_tb
            xf = xload.tile([128, 2, ntok], F32)
            xb = xcast.tile([128, 2, ntok], BF16)
            for k in range(2):
                nc.sync.dma_start(
                    out=xf[:, k, :], in_=xt[k, :, t0 * B:(t0 + proj_tb) * B],
                )
            nc.scalar.copy(xb[:, :, :], xf[:, :, :])
            for c in range(8):
                ps = ppsum.tile([128, ntok], F32)
                for k in range(2):
                    nc.tensor.matmul(
                        ps[:, :],
                        wb[:, k, c * 128:(c + 1) * 128],
                        xb[:, k, :],
                        start=(k == 0),
                        stop=(k == 1),
                    )
                dst = xw[:, t0:t0 + proj_tb, c * NB:(c + 1) * NB]
                if c < 2:  # cand chunks carry the bias
                    nc.vector.tensor_scalar(
                        dst, ps[:, :], bct[:, c:c + 1], None,
                        mybir.AluOpType.add,
                    )
                elif c % 2 == 0:
                    nc.scalar.copy(dst, ps[:, :])
                else:
                    nc.vector.tensor_copy(dst, ps[:, :])

        # state: cand (0:HW) | c (HW:2HW)
        state = const.tile([128, 2 * HW], F32)
        nc.vector.memset(state[:, :], 0.0)
        nseg = T // seg
        for si in range(nseg):
            hseg = hsegp.tile([128, seg, HW], BF16)
            for tl in range(seg):
                t = si * seg + tl
                g = gpsum.tile([128, 6 * NB], F32, tag="gcif")
                go = gpsum.tile([128, 2 * NB], F32, tag="go")
                nc.tensor.matmul(
                    g[:, :], ident[:, :], xw[:, t, :6 * NB],
                    start=True, stop=False, skip_group_check=True,
                )
                nc.tensor.matmul(
                    go[:, :], ident[:, :], xw[:, t, 6 * NB:],
                    start=True, stop=False, skip_group_check=True,
                )

                def rmm(c, k, stop=False):
                    rhs = (hzero[:, k * NB:(k + 1) * NB] if t == 0
                           else h2[:, (t - 1) if hfull else (t - 1) % 2, k * NB:(k + 1) * NB])
                    dst = (g[:, c * NB:(c + 1) * NB] if c < 6
                           else go[:, (c - 6) * NB:(c - 5) * NB])
                    nc.tensor.matmul(
                        dst,
                        rb[:, k, c * 128:(c + 1) * 128],
                        rhs,
                        start=False, stop=stop, skip_group_check=True,
                    )

                u = work.tile([128, W8], F32, tag="u")
                for c in range(6):
                    for k in range(2):
                        rmm(c, k)
                TANH = mybir.ActivationFunctionType.Tanh
                if split_sig:
                    if use_tanh:
                        # cand = tanh(g_c) straight into state[:, 0:HW]
                        nc.scalar.activation(state[:, :HW], g[:, :2 * NB],
                                             TANH, scale=0.5)
                        nc.scalar.activation(u[:, HW:6 * NB], g[:, 2 * NB:], SIG)
                    else:
                        nc.scalar.activation(u[:, :6 * NB], g[:, :], SIG)
                for c in (6, 7):
                    for k in range(2):
                        rmm(c, k, stop=(c == 7 and k == 1))
                if split_sig:
                    nc.scalar.activation(u[:, 6 * NB:], go[:, :], SIG)
                else:
                    nc.scalar.activation(u[:, :6 * NB], g[:, :], SIG)
                    nc.scalar.activation(u[:, 6 * NB:], go[:, :], SIG)
                if not use_tanh:
                    # cand = 2*u_c - 1 -> state[:, 0:HW]
                    nc.vector.tensor_scalar(
                        state[:, :HW], u[:, :HW], 2.0, -1.0,
                        mybir.AluOpType.mult, mybir.AluOpType.add,
                    )
                # prod = [i|f] * [cand|c] -> [m2|m1]
                prod = work.tile([128, 4 * HW], F32, tag="prod")
                nc.vector.tensor_mul(
                    prod[:, :2 * HW], u[:, HW:3 * HW], state[:, :],
                )
                s = prod[:, 2 * HW:3 * HW]
                nc.vector.tensor_add(s, prod[:, :HW], prod[:, HW:2 * HW])
                phi = prod[:, 3 * HW:]
                nc.scalar.activation(phi, s, SIG, bias=bphi[:, :], scale=AL_PHI)
                nc.vector.tensor_mul(
                    h2[:, t if hfull else t % 2, :], phi, u[:, 6 * NB:],
                )
                nc.scalar.activation(state[:, HW:], s, SIG)
                nc.vector.tensor_mul(
                    hseg[:, tl, :], phi, u[:, 6 * NB:],
                )

            t0 = si * seg
            ost = ostage.tile([128, 2, seg, NB], F32)
            nc.scalar.activation(
                ost[:, :, :, :],
                hseg[:, :, :].rearrange("p t (k b) -> p k t b", k=2),
                COPY, scale=K_PHI,
            )
            for k in range(2):
                nc.sync.dma_start(
                    out=out[k, :, t0 * B:(t0 + seg) * B],
                    in_=ost[:, k, :, :],
                )
    _split_syncs(nc)
    return nc


def _prep_weights(Wd, Rd, bcd):
    # reference gate order [i f o c] -> kernel order [c i f o]
    perm = np.concatenate([
        np.arange(3 * U, 4 * U), np.arange(0, U),
        np.arange(U, 2 * U), np.arange(2 * U, 3 * U),
    ])
    Wp = np.ascontiguousarray(Wd[:, perm]).astype(np.float32)
    Rp = np.ascontiguousarray(Rd[:, perm]).astype(np.float32)
    Wp[:, :U] *= 2.0
    Rp[:, :U] *= 2.0
    Rp *= K_PHI
    bcg = np.ascontiguousarray((2.0 * bcd).reshape(2, 128).T).astype(np.float32)
    return Wp, Rp, bcg


def kernel(x, W_f, R_f, bc_f, W_b, R_b, bc_b):
    from concourse.bass_utils import run_bass_kernel_spmd

    x = np.asarray(x, dtype=np.float32)
    if "nc" not in _CACHE:
        _CACHE["nc"] = _build_v3()
    nc = _CACHE["nc"]

    Wf, Rf, bcgf = _prep_weights(np.asarray(W_f, np.float32),
                                 np.asarray(R_f, np.float32),
                                 np.asarray(bc_f, np.float32))
    Wb_, Rb_, bcgb = _prep_weights(np.asarray(W_b, np.float32),
                                   np.asarray(R_b, np.float32),
                                   np.asarray(bc_b, np.float32))

    in_maps = []
    for core in range(8):
        fwd = core < 4
        b0 = (core % 4) * BL
        xs = x[b0:b0 + BL]
        if not fwd:
            xs = xs[:, ::-1, :]
        xtr = np.ascontiguousarray(xs.transpose(2, 1, 0)).reshape(2, 128, T * BL)
        in_maps.append({
            "xt": xtr,
            "w": Wf if fwd else Wb_,
            "r": Rf if fwd else Rb_,
            "bcg": bcgf if fwd else bcgb,
        })

    res = run_bass_kernel_spmd(nc, in_maps, core_ids=list(range(8)))

    outp = np.empty((B, T, 2 * U), dtype=np.float32)
    for core in range(8):
        od = res.results[core]["out"]  # [2, 128, T*BL]
        hb = od.reshape(256, T, BL).transpose(2, 1, 0)  # [BL, T, U]
        b0 = (core % 4) * BL
        if core < 4:
            outp[b0:b0 + BL, :, 0:U] = hb
        else:
            outp[b0:b0 + BL, :, U:2 * U] = hb
    return outp
